# revision 3
# baseline (speedup 1.0000x reference)
"""Trainium2 Bass kernel for nn_DGM_layer_59313498357972.

DGM (Deep Galerkin Method) loss: forward + input-gradient (backward) passes of a
small gated residual network over 2048 interior points, 32x2048 MC-perturbed
points, and 2048 boundary points.

Strategy: pure data-parallel over 8 NeuronCores. Each core runs an identical
Tile program: one "xv" tile (256 interior + 256 boundary points: fwd + bwd +
value) and 16 "mc" tiles of 512 perturbed points each (fwd + bwd + the
viol-weighted gradient reduction). Activations are fp16 in SBUF
([128 units x 512 batch] layout), matmul accumulation fp32 in PSUM.
All weight transposes / bias-folding / sign-folding are done host-side.
The tiny final combine (drift dot-products, payoff, scaling) runs on host.
"""

import numpy as np

# ---- model constants (hardcoded; must match the reference problem) ----
DIM = 20
D21 = 21
D22 = 22
MC = 32
NPT = 2048          # interior (and boundary) point count
UNITS = 128
SIG = 0.2
MU = 0.05
RR = 0.05
DELTA = 0.01
RHO = 0.5

NCORES = 8
B = 512             # batch columns per tile
TM = (MC * NPT) // (NCORES * B)   # 16 mc tiles per core
NX = NPT // NCORES                # 256 interior points per core

_CACHE = {}


def _l_tril():
    cov = DELTA * (np.full((DIM, DIM), RHO) + (1.0 - RHO) * np.eye(DIM))
    return np.linalg.cholesky(cov).astype(np.float32)


def _build():
    """Build + compile the per-core Bass/Tile program once per process."""
    if "nc" in _CACHE:
        return _CACHE

    import concourse.mybir as mybir
    import concourse.tile as tile
    from concourse import bacc
    from concourse.alu_op_type import AluOpType as op

    f16 = mybir.dt.float16
    f32 = mybir.dt.float32
    A = mybir.ActivationFunctionType

    nc = bacc.Bacc("TRN2", target_bir_lowering=False, debug=False,
                   num_devices=NCORES)

    # ---- DRAM I/O ----
    wspecs = {
        # x-side weights, bias folded in as last row (rhs has a ones row)
        "ua_z": [D22, UNITS], "ua_g": [D22, UNITS], "ua_r": [D22, UNITS],
        "ua_h": [D22, UNITS], "w1a": [D22, UNITS],
        # s-side weights (lhsT = W: out[v,b] = sum_u W[u,v] s[u,b])
        "wz": [UNITS, UNITS], "wg": [UNITS, UNITS],
        "wr": [UNITS, UNITS], "wh": [UNITS, UNITS],
        # backward (transposed; g-branch negated for the -h factor sign)
        "wzT": [UNITS, UNITS], "wgTn": [UNITS, UNITS],
        "wrT": [UNITS, UNITS], "whT": [UNITS, UNITS],
        # dx accumulation weights
        "uzT": [UNITS, D21], "ugTn": [UNITS, D21], "urT": [UNITS, D21],
        "uhT": [UNITS, D21], "w1T": [UNITS, D21],
        # misc
        "wvec": [UNITS, 1], "ones21": [D21, 1], "ds3": [UNITS, B],
    }
    dr = {}
    for name, shape in wspecs.items():
        dr[name] = nc.dram_tensor(name, shape, f16, kind="ExternalInput")
    xt_mc_d = nc.dram_tensor("xt_mc", [TM, D22, B], f16, kind="ExternalInput")
    vt_mc_d = nc.dram_tensor("vt_mc", [TM, D21, B], f16, kind="ExternalInput")
    xt_xv_d = nc.dram_tensor("xt_xv", [D22, B], f16, kind="ExternalInput")
    s1_out_d = nc.dram_tensor("s1_out", [TM, 1, B], f32, kind="ExternalOutput")
    dx_out_d = nc.dram_tensor("dx_out", [D21, B], f32, kind="ExternalOutput")
    val_out_d = nc.dram_tensor("val_out", [1, B], f32, kind="ExternalOutput")

    with tile.TileContext(nc) as tc:
        with (
            tc.tile_pool(name="const", bufs=1) as cp,
            tc.tile_pool(name="io", bufs=3) as iop,
            tc.tile_pool(name="sv", bufs=2) as svp,
            tc.tile_pool(name="tp", bufs=2) as tpp,
            tc.tile_pool(name="psg", bufs=4, space="PSUM") as psg,
            tc.tile_pool(name="psb", bufs=2, space="PSUM") as psb,
            tc.tile_pool(name="psd", bufs=1, space="PSUM") as psd,
            tc.tile_pool(name="pso", bufs=1, space="PSUM") as pso,
        ):
            W = {}
            for name, shape in wspecs.items():
                t = cp.tile(list(shape), f16, tag=name)
                nc.sync.dma_start(t[:], dr[name].ap())
                W[name] = t

            def TT(dst, a, b_, o=op.mult):
                nc.vector.tensor_tensor(dst[:], a[:], b_[:], o)

            def tanh(dst, src):
                nc.scalar.activation(dst[:], src[:], A.Tanh)

            def do_tile(kind, t_idx):
                is_mc = kind == "mc"
                xt = iop.tile([D22, B], f16, tag="xt")
                if is_mc:
                    nc.sync.dma_start(xt[:], xt_mc_d.ap()[t_idx])
                    vt = iop.tile([D21, B], f16, tag="vt")
                    nc.sync.dma_start(vt[:], vt_mc_d.ap()[t_idx])
                else:
                    nc.sync.dma_start(xt[:], xt_xv_d.ap())

                def gate(ua, ws, rhs_t):
                    ps = psg.tile([UNITS, B], f32, tag="g")
                    nc.tensor.matmul(ps[:], W[ua][:], xt[:], start=True, stop=False)
                    nc.tensor.matmul(ps[:], W[ws][:], rhs_t[:], start=False, stop=True)
                    return ps

                # ---------- forward ----------
                ps0 = psg.tile([UNITS, B], f32, tag="g")
                nc.tensor.matmul(ps0[:], W["w1a"][:], xt[:], start=True, stop=True)
                s0 = svp.tile([UNITS, B], f16, tag="s0")
                tanh(s0, ps0)
                s_t = s0
                sv = {}
                for l in range(3):
                    ps_z = gate("ua_z", "wz", s_t)
                    ps_g = gate("ua_g", "wg", s_t)
                    ps_r = gate("ua_r", "wr", s_t)
                    z = svp.tile([UNITS, B], f16, tag=f"z{l}")
                    tanh(z, ps_z)
                    g = svp.tile([UNITS, B], f16, tag=f"g{l}")
                    tanh(g, ps_g)
                    r = svp.tile([UNITS, B], f16, tag=f"r{l}")
                    tanh(r, ps_r)
                    og = svp.tile([UNITS, B], f16, tag=f"og{l}")
                    nc.vector.tensor_scalar(og[:], g[:], -1.0, 1.0, op.mult, op.add)
                    sr = tpp.tile([UNITS, B], f16, tag="sr")
                    TT(sr, s_t, r)
                    ps_h = gate("ua_h", "wh", sr)
                    h = svp.tile([UNITS, B], f16, tag=f"h{l}")
                    tanh(h, ps_h)
                    sv[l] = dict(s=s_t, z=z, g=g, r=r, og=og, h=h)
                    if l < 2 or not is_mc:
                        m1 = tpp.tile([UNITS, B], f16, tag="m1")
                        TT(m1, og, h)
                        q = tpp.tile([UNITS, B], f16, tag="q")
                        TT(q, z, s_t)
                        s_t = svp.tile([UNITS, B], f16, tag=f"s{l + 1}")
                        TT(s_t, m1, q, op.add)

                # ---------- backward (grad of sum(val) wrt x) ----------
                dx = psd.tile([D21, B], f32, tag="dx")
                ds = W["ds3"]
                first_dx = [True]

                def dxmm(wname, da):
                    nc.tensor.matmul(dx[:], W[wname][:], da[:],
                                     start=first_dx[0], stop=False)
                    first_dx[0] = False

                def chain(y, tag, t_src, t_with):
                    # returns da = (t_src * t_with) * (1 - y*y)
                    # (t_with=None: da = t_src * (1 - y*y))
                    sq = tpp.tile([UNITS, B], f16, tag=f"sq{tag}")
                    nc.scalar.square(sq[:], y[:])
                    f = tpp.tile([UNITS, B], f16, tag=f"f{tag}")
                    nc.vector.tensor_scalar(f[:], sq[:], -1.0, 1.0, op.mult, op.add)
                    if t_with is not None:
                        t = tpp.tile([UNITS, B], f16, tag=f"t{tag}")
                        TT(t, t_src, t_with)
                    else:
                        t = t_src
                    da = tpp.tile([UNITS, B], f16, tag=f"da{tag}")
                    TT(da, t, f)
                    return da

                for l in (2, 1, 0):
                    vv = sv[l]
                    daz = chain(vv["z"], "z", ds, vv["s"])
                    dag = chain(vv["g"], "g", ds, vv["h"])   # sign in wgTn/ugTn
                    dah = chain(vv["h"], "h", ds, vv["og"])
                    ps_dsr = psb.tile([UNITS, B], f32, tag="b")
                    nc.tensor.matmul(ps_dsr[:], W["whT"][:], dah[:],
                                     start=True, stop=True)
                    dsr = tpp.tile([UNITS, B], f16, tag="dsr")
                    nc.scalar.copy(dsr[:], ps_dsr[:])
                    dar = chain(vv["r"], "r", dsr, vv["s"])
                    ps_ds = psb.tile([UNITS, B], f32, tag="b")
                    nc.tensor.matmul(ps_ds[:], W["wzT"][:], daz[:],
                                     start=True, stop=False)
                    nc.tensor.matmul(ps_ds[:], W["wgTn"][:], dag[:],
                                     start=False, stop=False)
                    nc.tensor.matmul(ps_ds[:], W["wrT"][:], dar[:],
                                     start=False, stop=True)
                    dxmm("uzT", daz)
                    dxmm("ugTn", dag)
                    dxmm("urT", dar)
                    dxmm("uhT", dah)
                    m1b = tpp.tile([UNITS, B], f16, tag="m1b")
                    TT(m1b, ds, vv["z"])
                    m2b = tpp.tile([UNITS, B], f16, tag="m2b")
                    TT(m2b, dsr, vv["r"])
                    u = tpp.tile([UNITS, B], f16, tag="u")
                    TT(u, m1b, m2b, op.add)
                    nds = tpp.tile([UNITS, B], f16, tag="ds")
                    TT(nds, u, ps_ds, op.add)
                    ds = nds

                dp1 = chain(s0, "p1", ds, None)  # special: dp1 = ds*(1-s0^2)
                nc.tensor.matmul(dx[:], W["w1T"][:], dp1[:], start=False, stop=True)

                if is_mc:
                    prod = tpp.tile([D21, B], f16, tag="prod")
                    TT(prod, vt, dx)
                    ps1 = pso.tile([1, B], f32, tag="o")
                    nc.tensor.matmul(ps1[:], W["ones21"][:], prod[:],
                                     start=True, stop=True)
                    s1s = tpp.tile([1, B], f32, tag="s1s")
                    nc.scalar.copy(s1s[:], ps1[:])
                    nc.sync.dma_start(s1_out_d.ap()[t_idx], s1s[:])
                else:
                    dxo = tpp.tile([D21, B], f32, tag="dxo")
                    nc.scalar.copy(dxo[:], dx[:])
                    nc.sync.dma_start(dx_out_d.ap(), dxo[:])
                    psv = pso.tile([1, B], f32, tag="o")
                    nc.tensor.matmul(psv[:], W["wvec"][:], s_t[:],
                                     start=True, stop=True)
                    vo = tpp.tile([1, B], f32, tag="vo")
                    nc.scalar.copy(vo[:], psv[:])
                    nc.sync.dma_start(val_out_d.ap(), vo[:])

            do_tile("xv", 0)
            for t in range(TM):
                do_tile("mc", t)

    nc.compile()
    _CACHE["nc"] = nc
    return _CACHE


def _host_prep(inputs):
    """Quantize, shard, transpose inputs; returns (in_maps, host_ctx)."""
    f16 = np.float16
    x = np.asarray(inputs["inputs"], np.float32)
    eps = np.asarray(inputs["eps"], np.float32)
    x1, x2 = x[:NPT], x[NPT:]
    L = _l_tril()
    loc = x1[:, :DIM]
    sample = loc[None] + np.einsum("mnd,kd->mnk", eps, L, optimize=True)
    viol = np.concatenate(
        [sample * (SIG * loc)[None], np.zeros((MC, NPT, 1), np.float32)], axis=2
    )
    viol16 = viol.astype(f16)
    xplus16 = (x1[None] + viol16.astype(np.float32)).astype(f16)

    # mc shards: flat index = mc*NPT + n, contiguous blocks of TM*B per core
    xf = xplus16.reshape(MC * NPT, D21)
    vf = viol16.reshape(MC * NPT, D21)
    per_core = MC * NPT // NCORES

    def aug_ones(xT):  # [21, n] -> [22, n] with ones row
        return np.concatenate([xT, np.ones((1, xT.shape[1]), f16)], 0)

    p = {k: np.asarray(v, np.float32) for k, v in inputs.items()}
    wz, wg, wr, wh = p["wzl"], p["wgl"], p["wrl"], p["whl"]
    shared = {
        "ua_z": np.vstack([p["uzl"], p["bzl"]]).astype(f16),
        "ua_g": np.vstack([p["ugl"], p["bgl"]]).astype(f16),
        "ua_r": np.vstack([p["url"], p["brl"]]).astype(f16),
        "ua_h": np.vstack([p["uhl"], p["bhl"]]).astype(f16),
        "w1a": np.vstack([p["w1"], p["b1"]]).astype(f16),
        "wz": wz.astype(f16), "wg": wg.astype(f16),
        "wr": wr.astype(f16), "wh": wh.astype(f16),
        "wzT": wz.T.copy().astype(f16), "wgTn": (-wg.T).copy().astype(f16),
        "wrT": wr.T.copy().astype(f16), "whT": wh.T.copy().astype(f16),
        "uzT": p["uzl"].T.copy().astype(f16),
        "ugTn": (-p["ugl"].T).copy().astype(f16),
        "urT": p["url"].T.copy().astype(f16),
        "uhT": p["uhl"].T.copy().astype(f16),
        "w1T": p["w1"].T.copy().astype(f16),
        "wvec": p["w"].astype(f16),
        "ones21": np.ones((D21, 1), f16),
        "ds3": np.repeat(p["w"].astype(f16), B, axis=1),
    }

    in_maps = []
    for c in range(NCORES):
        xc = xf[c * per_core:(c + 1) * per_core]      # [8192, 21]
        vc = vf[c * per_core:(c + 1) * per_core]
        xt_mc = np.ascontiguousarray(
            np.stack([aug_ones(xc[t * B:(t + 1) * B].T) for t in range(TM)]))
        vt_mc = np.ascontiguousarray(
            np.stack([vc[t * B:(t + 1) * B].T for t in range(TM)]))
        xv = np.concatenate([x1[c * NX:(c + 1) * NX], x2[c * NX:(c + 1) * NX]], 0)
        xt_xv = np.ascontiguousarray(aug_ones(xv.T.astype(f16)))
        m = dict(shared)
        m["xt_mc"] = xt_mc
        m["vt_mc"] = vt_mc
        m["xt_xv"] = xt_xv
        in_maps.append(m)

    ctx = dict(x1=x1.astype(np.float64), x2=x2.astype(np.float64),
               viol16=viol16, b=float(np.asarray(inputs["b"]).reshape(-1)[0]))
    return in_maps, ctx


def _combine(results, ctx):
    x1, x2 = ctx["x1"], ctx["x2"]
    bb = ctx["b"]
    fprime = np.empty((NPT, D21))
    val1 = np.empty(NPT)
    val2 = np.empty(NPT)
    s1 = np.empty(MC * NPT)
    per_core = MC * NPT // NCORES
    for c in range(NCORES):
        r = results[c]
        fprime[c * NX:(c + 1) * NX] = np.asarray(r["dx_out"], np.float64)[:, :NX].T
        v = np.asarray(r["val_out"], np.float64)[0]
        val1[c * NX:(c + 1) * NX] = v[:NX] + bb
        val2[c * NX:(c + 1) * NX] = v[NX:] + bb
        s1[c * per_core:(c + 1) * per_core] = np.asarray(
            r["s1_out"], np.float64).reshape(-1)
    s1 = s1.reshape(MC, NPT)
    V = ctx["viol16"].astype(np.float64).sum(0)
    term1_2 = (s1.sum(0) - (fprime * V).sum(1)) / (MC * DELTA)
    loc = x1[:, :DIM]
    drift = np.concatenate([MU * loc, np.ones((NPT, 1))], 1)
    term1_1 = (drift * fprime).sum(1)
    interior = term1_1 + 0.5 * term1_2 - RR * val1
    payoff = np.maximum(np.prod(x2[:, :DIM], axis=1) ** (1.0 / DIM), 0.0)
    term2 = val2 - payoff
    return np.concatenate([interior, term2]).astype(np.float32)


def run_device(in_maps):
    from concourse import bass_utils
    c = _build()
    res = bass_utils.run_bass_kernel_spmd(c["nc"], in_maps,
                                          core_ids=list(range(NCORES)))
    return res.results


def kernel(**inputs):
    in_maps, ctx = _host_prep(inputs)
    results = run_device(in_maps)
    return _combine(results, ctx)


# revision 29
# speedup vs baseline: 898.2933x; 898.2933x over previous
"""Trainium2 Bass kernel for nn_DGM_layer_59313498357972.

DGM (Deep Galerkin Method) loss: forward + input-gradient (backward) passes of
a small gated residual network over 2048 interior points, 32x2048 MC-perturbed
points, and 2048 boundary points.

Strategy: pure data-parallel over 8 NeuronCores. Each core runs an identical
Tile program: one "xv" tile (256 interior + 256 boundary points: fwd + bwd +
value) and 16 "mc" tiles of 512 perturbed points each (fwd + bwd + the
viol-weighted gradient reduction). Activations are fp16 in SBUF
([128 units x 512 batch] layout), matmul accumulation fp32 in PSUM.

Elementwise fusion: each layer's saved activations live in one
[128, 6*512] "hexa" tile ordered [s|og|z|h|g|r] so that
  - one ScalarE Square covers [z|h|g|r] (the four tanh-grad squares),
  - one VectorE tensor_scalar folds all four (1 - y^2),
  - one VectorE tensor_tensor computes all four ds-products
    ([t_z|t_h|m1b|t_g] = ds * [s|og|z|h]) via a broadcast access pattern.
The last layer's incoming gradient is the constant vector w, which is folded
into host-scaled transposed weights (no t-products at all for that layer).
All weight transposes / bias-folding / sign-folding are done host-side.
The tiny final combine (drift dot-products, payoff, scaling) runs on host.
"""

import numpy as np

# ---- model constants (hardcoded; must match the reference problem) ----
DIM = 20
D21 = 21
D22 = 22
MC = 32
NPT = 2048          # interior (and boundary) point count
UNITS = 128
SIG = 0.2
MU = 0.05
RR = 0.05
DELTA = 0.01
RHO = 0.5

NCORES = 8
B = 512             # batch columns per tile
TM = (MC * NPT) // (NCORES * B)   # 16 mc tiles per core
NX = NPT // NCORES                # 256 interior points per core

# hexa slot offsets (elements): [s | og | z | h | g | r]
S_, OG_, Z_, H_, G_, R_ = 0, B, 2 * B, 3 * B, 4 * B, 5 * B

_CACHE = {}
L2SPEC = True
MODE = "phase"    # "layer": fine-grained fwd/bwd interleave; else phase
PIPE = 2          # (phase mode) fwd tiles emitted ahead of each bwd
SV_BUFS = 3
PSG, PSB = 3, 3
LOOP_REPS = 0     # timing builds: wrap the whole schedule in a HW loop


def _l_tril():
    cov = DELTA * (np.full((DIM, DIM), RHO) + (1.0 - RHO) * np.eye(DIM))
    return np.linalg.cholesky(cov).astype(np.float32)


def _build():
    """Build + compile the per-core Bass/Tile program once per process."""
    if "nc" in _CACHE:
        return _CACHE

    import concourse.mybir as mybir
    import concourse.tile as tile
    from concourse import bacc
    from concourse.alu_op_type import AluOpType as op

    f16 = mybir.dt.float16
    f32 = mybir.dt.float32
    A = mybir.ActivationFunctionType

    nc = bacc.Bacc("TRN2", target_bir_lowering=False, debug=False,
                   num_devices=NCORES)

    wspecs = {
        # x-side weights, bias folded in as last row (rhs has a ones row)
        "ua_z": [D22, UNITS], "ua_g": [D22, UNITS], "ua_r": [D22, UNITS],
        "ua_h": [D22, UNITS], "w1a": [D22, UNITS],
        # s-side weights (lhsT = W: out[v,b] = sum_u W[u,v] s[u,b])
        "wz": [UNITS, UNITS], "wg": [UNITS, UNITS],
        "wr": [UNITS, UNITS], "wh": [UNITS, UNITS],
        # backward (transposed; g-branch negated for the -h factor sign)
        "wzT": [UNITS, UNITS], "wgTn": [UNITS, UNITS],
        "wrT": [UNITS, UNITS], "whT": [UNITS, UNITS],
        # dx accumulation weights
        "uzT": [UNITS, D21], "ugTn": [UNITS, D21], "urT": [UNITS, D21],
        "uhT": [UNITS, D21], "w1T": [UNITS, D21],
        # layer-2 specialization: the same, pre-scaled by w (row-wise)
        "wzT_s": [UNITS, UNITS], "wgTn_s": [UNITS, UNITS],
        "whT_s": [UNITS, UNITS],
        "uzT_s": [UNITS, D21], "ugTn_s": [UNITS, D21], "uhT_s": [UNITS, D21],
        # misc
        "wvec": [UNITS, 1], "ones21": [D21, 1], "ds3": [UNITS, B],
    }
    dr = {}
    for name, shape in wspecs.items():
        dr[name] = nc.dram_tensor(name, shape, f16, kind="ExternalInput")
    dr["wvec32"] = nc.dram_tensor("wvec32", [UNITS, 1], f32,
                                  kind="ExternalInput")
    xt_mc_d = nc.dram_tensor("xt_mc", [TM, D22, B], f16, kind="ExternalInput")
    vt_mc_d = nc.dram_tensor("vt_mc", [TM, D21, B], f16, kind="ExternalInput")
    xt_xv_d = nc.dram_tensor("xt_xv", [D22, B], f16, kind="ExternalInput")
    s1_out_d = nc.dram_tensor("s1_out", [TM, 1, B], f32, kind="ExternalOutput")
    dx_out_d = nc.dram_tensor("dx_out", [D21, B], f32, kind="ExternalOutput")
    val_out_d = nc.dram_tensor("val_out", [1, B], f32, kind="ExternalOutput")

    with tile.TileContext(nc) as tc:
        with (
            tc.tile_pool(name="const", bufs=1) as cp,
            tc.tile_pool(name="io", bufs=5) as iop,
            tc.tile_pool(name="sv", bufs=SV_BUFS) as svp,
            tc.tile_pool(name="tp", bufs=2) as tpp,
            tc.tile_pool(name="psg", bufs=PSG, space="PSUM") as psg,
            tc.tile_pool(name="psb", bufs=PSB, space="PSUM") as psb,
            tc.tile_pool(name="psd", bufs=1, space="PSUM") as psd,
            tc.tile_pool(name="pso", bufs=1, space="PSUM") as pso,
        ):
            W = {}
            for name, shape in wspecs.items():
                t = cp.tile(list(shape), f16, tag=name)
                nc.sync.dma_start(t[:], dr[name].ap())
                W[name] = t
            wv32 = cp.tile([UNITS, 1], f32, tag="wvec32")
            nc.sync.dma_start(wv32[:], dr["wvec32"].ap())

            def TT(dst, a, b_, o=op.mult):
                nc.vector.tensor_tensor(dst, a, b_, o)

            def pairs(ap3072, off, n):
                """n consecutive B-slots of a hexa-like AP starting at off,
                as a [128, n, B] strided view."""
                return ap3072[:, off:off + n * B].rearrange(
                    "p (a n) -> p a n", a=n)

            def do_fwd(kind, t_idx):
                # generator: yields after each layer (for emission interleave)
                is_mc = kind == "mc"
                vt = None
                xt = iop.tile([D22, B], f16, tag="xt")
                if is_mc:
                    nc.sync.dma_start(xt[:], xt_mc_d.ap()[t_idx])
                    vt = iop.tile([D21, B], f16, tag="vt")
                    nc.sync.dma_start(vt[:], vt_mc_d.ap()[t_idx])
                else:
                    nc.sync.dma_start(xt[:], xt_xv_d.ap())

                def gate(ua, ws, rhs):
                    ps = psg.tile([UNITS, B], f32, tag="g")
                    nc.tensor.matmul(ps[:], W[ua][:], xt[:], start=True, stop=False)
                    nc.tensor.matmul(ps[:], W[ws][:], rhs, start=False, stop=True)
                    return ps

                # ---------- forward ----------
                ps0 = psg.tile([UNITS, B], f32, tag="g")
                nc.tensor.matmul(ps0[:], W["w1a"][:], xt[:], start=True, stop=True)
                hx = [svp.tile([UNITS, 6 * B], f16, tag=f"hexa{l}",
                               name=f"hexa{l}") for l in range(3)]
                nc.scalar.activation(hx[0][:, S_:S_ + B], ps0[:], A.Tanh)
                s3 = None
                fs = []
                for l in range(3):
                    h6 = hx[l]
                    s_ap = h6[:, S_:S_ + B]
                    ps_z = gate("ua_z", "wz", s_ap)
                    ps_g = gate("ua_g", "wg", s_ap)
                    ps_r = gate("ua_r", "wr", s_ap)
                    nc.scalar.activation(h6[:, Z_:Z_ + B], ps_z[:], A.Tanh)
                    nc.scalar.activation(h6[:, G_:G_ + B], ps_g[:], A.Tanh)
                    nc.scalar.activation(h6[:, R_:R_ + B], ps_r[:], A.Tanh)
                    nc.vector.tensor_scalar(h6[:, OG_:OG_ + B], h6[:, G_:G_ + B],
                                            -1.0, 1.0, op.mult, op.add)
                    sr = tpp.tile([UNITS, B], f16, tag="sr")
                    TT(sr[:], s_ap, h6[:, R_:R_ + B])
                    ps_h = gate("ua_h", "wh", sr[:])
                    nc.scalar.activation(h6[:, H_:H_ + B], ps_h[:], A.Tanh)
                    if l < 2 or not is_mc:
                        m1 = tpp.tile([UNITS, B], f16, tag="m1")
                        TT(m1[:], h6[:, OG_:OG_ + B], h6[:, H_:H_ + B])
                        q = tpp.tile([UNITS, B], f16, tag="q")
                        TT(q[:], h6[:, Z_:Z_ + B], s_ap)
                        if l < 2:
                            s_next = hx[l + 1][:, S_:S_ + B]
                        else:
                            s3 = tpp.tile([UNITS, B], f16, tag="s3")
                            s_next = s3[:]
                        TT(s_next, m1[:], q[:], op.add)
                    # tanh-grad factors f = [f_z|f_h|f_g|f_r] (used by bwd)
                    sq = tpp.tile([UNITS, 4 * B], f16, tag=f"sq{l}")
                    nc.scalar.square(sq[:], h6[:, Z_:Z_ + 4 * B])
                    fl = svp.tile([UNITS, 4 * B], f16, tag=f"f{l}",
                                  name=f"f{l}")
                    nc.vector.tensor_scalar(fl[:], sq[:], -1.0, 1.0,
                                            op.mult, op.add)
                    fs.append(fl)
                    if l < 2:
                        yield None
                sq0 = tpp.tile([UNITS, B], f16, tag="sq0")
                nc.scalar.square(sq0[:], hx[0][:, S_:S_ + B])
                f0 = svp.tile([UNITS, B], f16, tag="f0")
                nc.vector.tensor_scalar(f0[:], sq0[:], -1.0, 1.0,
                                        op.mult, op.add)
                st = dict(kind=kind, t_idx=t_idx, hx=hx, fs=fs, f0=f0,
                          s3=s3, vt=vt)
                yield st

            def do_bwd(st):
                kind, t_idx = st["kind"], st["t_idx"]
                hx, fs, f0, s3, vt = (st["hx"], st["fs"], st["f0"],
                                      st["s3"], st["vt"])
                is_mc = kind == "mc"
                # ---------- backward (grad of sum(val) wrt x) ----------
                dx = psd.tile([D21, B], f32, tag="dx")
                first_dx = [True]

                def dxmm(wname, da):
                    nc.tensor.matmul(dx[:], W[wname][:], da,
                                     start=first_dx[0], stop=False)
                    first_dx[0] = False

                # fp16 [128, B] tile; None means "constant w" (l=2)
                ds = None if L2SPEC else W["ds3"]
                for l in (2, 1, 0):
                    h6 = hx[l]
                    f = fs[l]
                    if ds is None:
                        # layer 2: incoming grad is the constant w (folded
                        # into the *_s weights). c = [c_z|c_h], c_g.
                        cpair = tpp.tile([UNITS, 2 * B], f16, tag="cpair")
                        TT(pairs(cpair[:], 0, 2), pairs(h6[:], S_, 2),
                           pairs(f[:], 0, 2))
                        cg = tpp.tile([UNITS, B], f16, tag="cg")
                        TT(cg[:], h6[:, H_:H_ + B], f[:, 2 * B:3 * B])
                        daz, dah, dag = (cpair[:, 0:B], cpair[:, B:2 * B],
                                         cg[:])
                        wsfx = "_s"
                        m1b = tpp.tile([UNITS, B], f16, tag="m1b")
                        nc.vector.tensor_scalar(m1b[:], h6[:, Z_:Z_ + B],
                                                wv32[:], None, op.mult)
                    else:
                        # t-quad: [t_z|t_h|m1b|t_g] = ds * [s|og|z|h]
                        tq = tpp.tile([UNITS, 4 * B], f16, tag="tq")
                        TT(pairs(tq[:], 0, 4),
                           ds[:].rearrange("p (a n) -> p a n", a=1)
                               .broadcast_to([UNITS, 4, B]),
                           pairs(h6[:], S_, 4))
                        dap = tpp.tile([UNITS, 2 * B], f16, tag="dap")
                        TT(pairs(dap[:], 0, 2), pairs(tq[:], 0, 2),
                           pairs(f[:], 0, 2))
                        dg = tpp.tile([UNITS, B], f16, tag="dg")
                        TT(dg[:], tq[:, 3 * B:4 * B], f[:, 2 * B:3 * B])
                        daz, dah, dag = dap[:, 0:B], dap[:, B:2 * B], dg[:]
                        wsfx = ""
                        m1b = tq[:, 2 * B:3 * B]

                    ps_dsr = psb.tile([UNITS, B], f32, tag="b")
                    nc.tensor.matmul(ps_dsr[:], W["whT" + wsfx][:], dah,
                                     start=True, stop=True)
                    dsr = tpp.tile([UNITS, B], f16, tag="dsr")
                    nc.scalar.copy(dsr[:], ps_dsr[:])
                    # [t_r|m2b] = dsr * [s|r]
                    trm = tpp.tile([UNITS, 2 * B], f16, tag="trm")
                    h6v = h6[:].rearrange("p (a n) -> p a n", a=6)
                    TT(pairs(trm[:], 0, 2),
                       dsr[:].rearrange("p (a n) -> p a n", a=1)
                             .broadcast_to([UNITS, 2, B]),
                       h6v[:, 0::5, :])
                    dar = tpp.tile([UNITS, B], f16, tag="dar")
                    TT(dar[:], trm[:, 0:B], f[:, 3 * B:4 * B])

                    ps_ds = psb.tile([UNITS, B], f32, tag="b")
                    nc.tensor.matmul(ps_ds[:], W["wzT" + wsfx][:], daz,
                                     start=True, stop=False)
                    nc.tensor.matmul(ps_ds[:], W["wgTn" + wsfx][:], dag,
                                     start=False, stop=False)
                    nc.tensor.matmul(ps_ds[:], W["wrT"][:], dar[:],
                                     start=False, stop=True)
                    dxmm("uzT" + wsfx, daz)
                    dxmm("ugTn" + wsfx, dag)
                    dxmm("urT", dar[:])
                    dxmm("uhT" + wsfx, dah)
                    u = tpp.tile([UNITS, B], f16, tag="u")
                    TT(u[:], m1b, trm[:, B:2 * B], op.add)
                    nds = tpp.tile([UNITS, B], f16, tag="ds")
                    TT(nds[:], u[:], ps_ds[:], op.add)
                    ds = nds
                    if l > 0:
                        yield None

                # dp1 = ds * (1 - s0^2)
                dp1 = tpp.tile([UNITS, B], f16, tag="dp1")
                TT(dp1[:], ds[:], f0[:])
                nc.tensor.matmul(dx[:], W["w1T"][:], dp1[:],
                                 start=False, stop=True)

                if is_mc:
                    prod = tpp.tile([D21, B], f16, tag="prod")
                    TT(prod[:], vt[:], dx[:])
                    ps1 = pso.tile([1, B], f32, tag="o")
                    nc.tensor.matmul(ps1[:], W["ones21"][:], prod[:],
                                     start=True, stop=True)
                    s1s = tpp.tile([1, B], f32, tag="s1s")
                    nc.scalar.copy(s1s[:], ps1[:])
                    nc.sync.dma_start(s1_out_d.ap()[t_idx], s1s[:])
                else:
                    dxo = tpp.tile([D21, B], f32, tag="dxo")
                    nc.scalar.copy(dxo[:], dx[:])
                    nc.sync.dma_start(dx_out_d.ap(), dxo[:])
                    psv = pso.tile([1, B], f32, tag="o")
                    nc.tensor.matmul(psv[:], W["wvec"][:], s3[:],
                                     start=True, stop=True)
                    vo = tpp.tile([1, B], f32, tag="vo")
                    nc.scalar.copy(vo[:], psv[:])
                    nc.sync.dma_start(val_out_d.ap(), vo[:])

            # software pipeline: emit fwd(i+1) interleaved with bwd(i) so
            # each engine's in-order queue overlaps the two tiles' work
            def run_fwd(g):
                st = None
                while st is None:
                    st = next(g)
                return st

            def run_bwd(g):
                for _ in g:
                    pass

            import contextlib
            loop_cm = (tc.For_i(0, LOOP_REPS, 1) if LOOP_REPS
                       else contextlib.nullcontext())
            tiles = [("xv", 0)] + [("mc", t) for t in range(TM)]
            if True:
                loop_cm.__enter__()
            if MODE == "layer":
                st = run_fwd(do_fwd(*tiles[0]))
                for t in tiles[1:]:
                    fg, bg = do_fwd(*t), do_bwd(st)
                    stn, fdone, bdone = None, False, False
                    while not (fdone and bdone):
                        if not fdone:
                            try:
                                v = next(fg)
                                if v is not None:
                                    stn, fdone = v, True
                            except StopIteration:
                                fdone = True
                        if not bdone:
                            try:
                                next(bg)
                            except StopIteration:
                                bdone = True
                    st = stn
                run_bwd(do_bwd(st))
            else:
                from collections import deque
                work = list(tiles)
                pend = deque()
                for _ in range(PIPE):
                    if work:
                        pend.append(run_fwd(do_fwd(*work.pop(0))))
                while pend:
                    nxt = run_fwd(do_fwd(*work.pop(0))) if work else None
                    run_bwd(do_bwd(pend.popleft()))
                    if nxt is not None:
                        pend.append(nxt)
            loop_cm.__exit__(None, None, None)

    nc.compile()
    _CACHE["nc"] = nc
    return _CACHE


def _host_prep(inputs):
    """Quantize, shard, transpose inputs; returns (in_maps, host_ctx)."""
    f16 = np.float16
    x = np.asarray(inputs["inputs"], np.float32)
    eps = np.asarray(inputs["eps"], np.float32)
    x1, x2 = x[:NPT], x[NPT:]
    L = _l_tril()
    loc = x1[:, :DIM]
    sample = loc[None] + np.einsum("mnd,kd->mnk", eps, L, optimize=True)
    viol = np.concatenate(
        [sample * (SIG * loc)[None], np.zeros((MC, NPT, 1), np.float32)], axis=2
    )
    viol16 = viol.astype(f16)
    xplus16 = (x1[None] + viol16.astype(np.float32)).astype(f16)

    # mc shards: flat index = mc*NPT + n, contiguous blocks of TM*B per core
    xf = xplus16.reshape(MC * NPT, D21)
    vf = viol16.reshape(MC * NPT, D21)
    per_core = MC * NPT // NCORES

    def aug_ones(xT):  # [21, n] -> [22, n] with ones row
        return np.concatenate([xT, np.ones((1, xT.shape[1]), f16)], 0)

    p = {k: np.asarray(v, np.float32) for k, v in inputs.items()}
    wz, wg, wr, wh = p["wzl"], p["wgl"], p["wrl"], p["whl"]
    w = p["w"]  # [128, 1]
    shared = {
        "ua_z": np.vstack([p["uzl"], p["bzl"]]).astype(f16),
        "ua_g": np.vstack([p["ugl"], p["bgl"]]).astype(f16),
        "ua_r": np.vstack([p["url"], p["brl"]]).astype(f16),
        "ua_h": np.vstack([p["uhl"], p["bhl"]]).astype(f16),
        "w1a": np.vstack([p["w1"], p["b1"]]).astype(f16),
        "wz": wz.astype(f16), "wg": wg.astype(f16),
        "wr": wr.astype(f16), "wh": wh.astype(f16),
        "wzT": wz.T.copy().astype(f16), "wgTn": (-wg.T).copy().astype(f16),
        "wrT": wr.T.copy().astype(f16), "whT": wh.T.copy().astype(f16),
        "uzT": p["uzl"].T.copy().astype(f16),
        "ugTn": (-p["ugl"].T).copy().astype(f16),
        "urT": p["url"].T.copy().astype(f16),
        "uhT": p["uhl"].T.copy().astype(f16),
        "w1T": p["w1"].T.copy().astype(f16),
        "wzT_s": (wz.T * w).astype(f16), "wgTn_s": (-wg.T * w).astype(f16),
        "whT_s": (wh.T * w).astype(f16),
        "uzT_s": (p["uzl"].T * w).astype(f16),
        "ugTn_s": (-p["ugl"].T * w).astype(f16),
        "uhT_s": (p["uhl"].T * w).astype(f16),
        "wvec": w.astype(f16),
        "wvec32": w.astype(np.float32),
        "ones21": np.ones((D21, 1), f16),
        "ds3": np.repeat(w.astype(f16), B, axis=1),
    }

    in_maps = []
    for c in range(NCORES):
        xc = xf[c * per_core:(c + 1) * per_core]      # [8192, 21]
        vc = vf[c * per_core:(c + 1) * per_core]
        xt_mc = np.ascontiguousarray(
            np.stack([aug_ones(xc[t * B:(t + 1) * B].T) for t in range(TM)]))
        vt_mc = np.ascontiguousarray(
            np.stack([vc[t * B:(t + 1) * B].T for t in range(TM)]))
        xv = np.concatenate([x1[c * NX:(c + 1) * NX], x2[c * NX:(c + 1) * NX]], 0)
        xt_xv = np.ascontiguousarray(aug_ones(xv.T.astype(f16)))
        m = dict(shared)
        m["xt_mc"] = xt_mc
        m["vt_mc"] = vt_mc
        m["xt_xv"] = xt_xv
        in_maps.append(m)

    ctx = dict(x1=x1.astype(np.float64), x2=x2.astype(np.float64),
               viol16=viol16, b=float(np.asarray(inputs["b"]).reshape(-1)[0]))
    return in_maps, ctx


def _combine(results, ctx):
    x1, x2 = ctx["x1"], ctx["x2"]
    bb = ctx["b"]
    fprime = np.empty((NPT, D21))
    val1 = np.empty(NPT)
    val2 = np.empty(NPT)
    s1 = np.empty(MC * NPT)
    per_core = MC * NPT // NCORES
    for c in range(NCORES):
        r = results[c]
        fprime[c * NX:(c + 1) * NX] = np.asarray(r["dx_out"], np.float64)[:, :NX].T
        v = np.asarray(r["val_out"], np.float64)[0]
        val1[c * NX:(c + 1) * NX] = v[:NX] + bb
        val2[c * NX:(c + 1) * NX] = v[NX:] + bb
        s1[c * per_core:(c + 1) * per_core] = np.asarray(
            r["s1_out"], np.float64).reshape(-1)
    s1 = s1.reshape(MC, NPT)
    V = ctx["viol16"].astype(np.float64).sum(0)
    term1_2 = (s1.sum(0) - (fprime * V).sum(1)) / (MC * DELTA)
    loc = x1[:, :DIM]
    drift = np.concatenate([MU * loc, np.ones((NPT, 1))], 1)
    term1_1 = (drift * fprime).sum(1)
    interior = term1_1 + 0.5 * term1_2 - RR * val1
    payoff = np.maximum(np.prod(x2[:, :DIM], axis=1) ** (1.0 / DIM), 0.0)
    term2 = val2 - payoff
    return np.concatenate([interior, term2]).astype(np.float32)


def run_device(in_maps):
    import time
    from concourse import bass_utils
    c = _build()
    last = None
    for attempt in range(4):
        try:
            res = bass_utils.run_bass_kernel_spmd(c["nc"], in_maps,
                                                  core_ids=list(range(NCORES)))
            return res.results
        except Exception as e:  # transient terminal/device hiccups
            last = e
            time.sleep(30 * (attempt + 1))
    raise last


def kernel(**inputs):
    in_maps, ctx = _host_prep(inputs)
    results = run_device(in_maps)
    return _combine(results, ctx)
